# revision 2
# baseline (speedup 1.0000x reference)
"""Trainium2 Bass kernel for nn_ActQuantWrapper (per-token 4-bit fake-quant + Linear).

Strategy (8 NeuronCores, SPMD, no collectives):
  - Shard x along the sequence axis: 1024 tokens per core; weight/bias replicated.
  - Host prep: features PERMUTED so the 3840 quantized features are columns
    [0:3840) and the 256 fp features are [3840:4096). The contraction
    mixed @ W^T is invariant under a common permutation of x-columns and
    W-columns, so no masks / copy_predicated / scatter are needed on device.
    W^T is pre-permuted and cast to bf16 on host.
  - Per core, per 128-token tile:
      * DVE: min/max reduces over the q-columns (then clamped with 0 to match
        the reference's min(.,0)/max(.,0)); per-token params in [128,1] tiles
      * ACT: round pass in place on the q-columns via the RNE +/-MAGIC trick
        fused into activation(x*inv + MAGIC)
      * DVE: clip (dual-op sub/min), then (max,mult) producing bf16 dq directly
        into mixed16[:, :3840]; fp columns copied as bf16(x) into [3840:4096)
      * DMA-xbar transpose (ACT queue) into mixed^T [feature, token] tiles
  - Matmul: stationary = mixed^T tile (128x128), moving = W^T chunk (N=512),
    PSUM accum over 32 feature tiles; DVE adds broadcast bias on drain;
    out DMA from the ACT queue. Two token groups (256 + 768 tokens): first
    group starts matmuls after only 2 tile-quants; W^T streams from HBM once
    per group (64 MiB total) with a 3-deep chunk prefetch on the sync queue.
  - DMA issue streams per engine: x + bias loads on GpSimd (SWDGE), W chunks
    on Sync (HWDGE), transposes + outputs on Scalar/ACT (HWDGE).
"""

import sys
import numpy as np
import ml_dtypes

sys.path.insert(0, "/opt/trn_rl_repo")

import concourse.bass as bass  # noqa: E402
import concourse.mybir as mybir  # noqa: E402
import concourse.tile as tile  # noqa: E402
from concourse import bacc  # noqa: E402

F32 = mybir.dt.float32
BF16 = mybir.dt.bfloat16

N_CORES = 8
S_FULL, D, O = 8192, 4096, 4096
DQ = 3840                      # quantized features, permuted to the front
T = S_FULL // N_CORES          # tokens per core
MAGIC = 12582912.0             # 1.5 * 2**23 : RNE round-to-int for |v| < 2**22
MAXQ = 15.0
RANGE_FLOOR = 1e-30            # degenerate all-zero token guard (dq ends up 0 anyway)

N_TT = T // 128                # token tiles per core
GROUP_TTS = [2, 6]             # token tiles per group (sums to N_TT)
CHUNK = 512                    # output-feature chunk per W^T stream tile
N_CH = O // CHUNK
N_DT = D // 128                # feature (contraction) tiles
MT_BUFS = 6                    # live mixed^T tiles
WC_BUFS = 3                    # W chunk prefetch depth

_CACHE = {}


def _build_bass():
    nc = bacc.Bacc("TRN2", target_bir_lowering=False, debug=False,
                   enable_asserts=True, num_devices=N_CORES)
    x_ap = nc.dram_tensor("x", [T, D], F32, kind="ExternalInput").ap()
    wt_ap = nc.dram_tensor("wt", [D, O], BF16, kind="ExternalInput").ap()
    bf_ap = nc.dram_tensor("biasf", [1, O], F32, kind="ExternalInput").ap()
    out_ap = nc.dram_tensor("out", [T, O], F32, kind="ExternalOutput").ap()

    with tile.TileContext(nc) as tc:
        _kernel_body(tc, out_ap, x_ap, wt_ap, bf_ap)
    nc.compile()
    return nc


def _kernel_body(tc, out_ap, x_ap, wt_ap, bf_ap):
    from contextlib import ExitStack
    nc = tc.nc
    A = mybir.AluOpType
    AF = mybir.ActivationFunctionType

    with ExitStack() as ctx:
        xp = ctx.enter_context(tc.tile_pool(name="xp", bufs=2))
        mxp = ctx.enter_context(tc.tile_pool(name="mxp", bufs=2))
        pp = ctx.enter_context(tc.tile_pool(name="pp", bufs=2))
        mtp = ctx.enter_context(tc.tile_pool(name="mtp", bufs=MT_BUFS))
        wcp = ctx.enter_context(tc.tile_pool(name="wcp", bufs=WC_BUFS))
        bbp = ctx.enter_context(tc.tile_pool(name="bbp", bufs=2))
        osp = ctx.enter_context(tc.tile_pool(name="osp", bufs=2))
        pmm = ctx.enter_context(tc.tile_pool(name="pmm", bufs=4, space="PSUM"))

        def load_wtc(ch):
            col = ch * CHUNK
            wtc = wcp.tile([128, N_DT, CHUNK], BF16, tag="wtc")
            nc.sync.dma_start(
                out=wtc,
                in_=wt_ap[0:D, col:col + CHUNK].rearrange("(j p) c -> p j c", p=128))
            bias_b = bbp.tile([128, CHUNK], F32, tag="bb")
            nc.gpsimd.dma_start(out=bias_b, in_=bass.AP(
                tensor=bf_ap.tensor, offset=bf_ap.offset + col,
                ap=[[0, 128], [1, CHUNK]]))
            return wtc, bias_b

        # chunk schedule: per group, all 8 W chunks; 16 loads total, 3 prefetched
        chunk_seq = [ch for _ in GROUP_TTS for ch in range(N_CH)]
        wtcs = {}
        for k in range(WC_BUFS):
            wtcs[k] = load_wtc(chunk_seq[k])

        mts = {}                      # global token-tile index -> mixed^T tile
        row0 = 0
        kseq = 0                      # position in chunk_seq
        for g, g_tts in enumerate(GROUP_TTS):
            for tt in range(g_tts):
                tti = row0 // 128 + tt
                row = row0 + tt * 128
                xt = xp.tile([128, D], F32, tag="x")
                nc.gpsimd.dma_start(out=xt, in_=x_ap[row:row + 128, :])
                xq = xt[:, 0:DQ]

                # per-token stats over q-columns, clamped with 0 (reference
                # uses min(qf.min, 0) / max(qf.max, 0))
                rmax = pp.tile([128, 1], F32, tag="rmax")
                rmin = pp.tile([128, 1], F32, tag="rmin")
                nc.vector.tensor_reduce(rmax, xq, axis=mybir.AxisListType.X, op=A.max)
                nc.vector.tensor_reduce(rmin, xq, axis=mybir.AxisListType.X, op=A.min)
                rmax0 = pp.tile([128, 1], F32, tag="rmax0")
                rmin0 = pp.tile([128, 1], F32, tag="rmin0")
                nc.vector.tensor_scalar(rmax0, rmax, 0.0, None, A.max)
                nc.vector.tensor_scalar(rmin0, rmin, 0.0, None, A.min)

                rng = pp.tile([128, 1], F32, tag="rng")
                nc.vector.tensor_tensor(rng, rmax0, rmin0, A.subtract)
                s = pp.tile([128, 1], F32, tag="s")       # scale = range/15
                nc.vector.tensor_scalar(s, rng, RANGE_FLOOR, 1.0 / MAXQ, A.max, A.mult)
                inv = pp.tile([128, 1], F32, tag="inv")
                nc.vector.reciprocal(inv, s)
                lop = pp.tile([128, 1], F32, tag="lop")   # lo = round(xmin/scale) = -zero
                nc.vector.tensor_scalar(lop, rmin0, inv, MAGIC, A.mult, A.add)
                lo = pp.tile([128, 1], F32, tag="lo")
                nc.vector.tensor_scalar(lo, lop, MAGIC, None, A.subtract)
                hi = pp.tile([128, 1], F32, tag="hi")
                nc.vector.tensor_scalar(hi, lo, MAXQ, None, A.add)

                # quantize in place on the q-columns:
                #   ACT: xq <- xq*inv + MAGIC   (RNE round-to-int in the mantissa)
                #   DVE: xq <- min(xq - MAGIC, hi) ; mixed16 <- bf16(max(xq, lo) * s)
                nc.scalar.activation(xq, xq, AF.Copy, bias=MAGIC, scale=inv)
                nc.vector.tensor_scalar(xq, xq, MAGIC, hi, A.subtract, A.min)
                mixed16 = mxp.tile([128, D], BF16, tag="mx")
                nc.vector.tensor_scalar(mixed16[:, 0:DQ], xq, lo, s, A.max, A.mult)
                # fp columns keep x (cast to bf16)
                nc.vector.tensor_copy(mixed16[:, DQ:D], xt[:, DQ:D])

                # DMA-xbar block-transpose: mt[p, j, t] = mixed16[t, 128*j + p]
                mt = mtp.tile([128, N_DT, 128], BF16, tag="mt")
                mts[tti] = mt
                nc.scalar.dma_start_transpose(mt, mixed16)

            # matmul phase: stream W^T chunks, accumulate over feature tiles
            for ch in range(N_CH):
                col = ch * CHUNK
                wtc, bias_b = wtcs.pop(kseq)
                if kseq + WC_BUFS < len(chunk_seq):
                    wtcs[kseq + WC_BUFS] = load_wtc(chunk_seq[kseq + WC_BUFS])
                kseq += 1

                for tt in range(g_tts):
                    tti = row0 // 128 + tt
                    row = row0 + tt * 128
                    ps = pmm.tile([128, CHUNK], F32, tag="mm")
                    for j in range(N_DT):
                        nc.tensor.matmul(ps, lhsT=mts[tti][:, j, :], rhs=wtc[:, j, :],
                                         start=(j == 0), stop=(j == N_DT - 1))
                    ost = osp.tile([128, CHUNK], F32, tag="ost")
                    nc.vector.tensor_tensor(ost, ps, bias_b, A.add)
                    nc.scalar.dma_start(out=out_ap[row:row + 128, col:col + CHUNK],
                                        in_=ost)
            row0 += g_tts * 128


def _get_nc():
    if "nc" not in _CACHE:
        _CACHE["nc"] = _build_bass()
    return _CACHE["nc"]


def _prep_in_maps(x, weight, bias, q_idx, fp_idx):
    x = np.ascontiguousarray(np.asarray(x, dtype=np.float32)).reshape(S_FULL, D)
    weight = np.asarray(weight, dtype=np.float32)
    bias = np.asarray(bias, dtype=np.float32)
    q_idx = np.asarray(q_idx).astype(np.int64)
    fp_idx = np.asarray(fp_idx).astype(np.int64)

    perm = np.concatenate([q_idx, fp_idx])
    xp = np.ascontiguousarray(x[:, perm])
    wt = np.ascontiguousarray(weight[:, perm].T.astype(ml_dtypes.bfloat16))

    shared = {"wt": wt, "biasf": np.ascontiguousarray(bias[None, :])}
    return [
        {"x": np.ascontiguousarray(xp[c * T:(c + 1) * T]), **shared}
        for c in range(N_CORES)
    ]


def kernel(x, weight, bias, q_idx, fp_idx):
    from concourse import bass_utils
    bass_utils.upload_artifacts = lambda tmpdir: "local://none"

    nc = _get_nc()
    in_maps = _prep_in_maps(x, weight, bias, q_idx, fp_idx)
    res = bass_utils.run_bass_kernel_spmd(
        nc, in_maps, core_ids=list(range(N_CORES)))
    out = np.concatenate([res.results[c]["out"] for c in range(N_CORES)], axis=0)
    return out.reshape(1, S_FULL, O)
